# revision 18
# baseline (speedup 1.0000x reference)
"""Self-contained TRN2 Bass kernel for nn_AdaptiveDenoisingQueryGenerator.

Data-parallel over batch: B=32 -> 4 batches/core x 8 cores. Selection-critical
arithmetic (FPS, score MLP, ranking) replicates XLA-CPU f32 bit-level; bulk
GEMMs run on the PE array (value-accurate).
"""
import sys
sys.path.insert(0, '/opt/trn_rl_repo')
import os
import struct
import numpy as np

import concourse.bass as bass
import concourse.bacc as bacc
import concourse.mybir as mybir
from concourse.tile import TileContext
from concourse.tile_rust import add_dep_helper
from concourse.bass_utils import run_bass_kernel_spmd

F32 = mybir.dt.float32
U32 = mybir.dt.uint32
AL = mybir.AluOpType
AF = mybir.ActivationFunctionType
AX = mybir.AxisListType


def _hexf(h):
    return np.float32(struct.unpack('>d', int(h, 16).to_bytes(8, 'big'))[0])

ERFC_C = [_hexf('0x3F1496A320000000'), _hexf('0xBF4A3F7000000000'),
          _hexf('0x3F75405B20000000'), _hexf('0xBF9B7F90E0000000'),
          _hexf('0x3FBCE2CF80000000'), _hexf('0xBFD81273E0000000'),
          _hexf('0x3FF20DD740000000')]
INV_SQRT2 = _hexf('0x3FE6A09E60000000')
EXP_CS = [_hexf('0x3F2A0D2CE0000000'), _hexf('0x3F56E879C0000000'),
          _hexf('0x3F81112100000000'), _hexf('0x3FA5553820000000'),
          _hexf('0x3FC5555540000000')]

NB = 4
NQ = 512
NBANK = 768
NROWS = NB * NBANK        # 3072
NRT = NROWS // 128        # 24
DEBUG = os.environ.get('KERNEL_DEBUG', '0') == '1'
FPS_ITERS = int(os.environ.get('KERNEL_FPS_ITERS', '256'))


def build(nc):
    ins = {}
    def P(name, shape):
        ins[name] = nc.declare_dram_parameter(name, list(shape), F32, isOutput=False)
    P('encoded_features', (NB, 384, 2048))
    P('input_coords', (NB, 16384, 3))
    P('conv1_w', (1024, 384)); P('conv1_b', (1024,))
    P('bn_g', (1024,)); P('bn_b', (1024,)); P('bn_m', (1024,)); P('bn_v', (1024,))
    P('conv2_w', (1024, 1024)); P('conv2_b', (1024,))
    P('cp1_w', (1024, 1024)); P('cp1_b', (1024,))
    P('cp2_w', (1024, 1536)); P('cp2_b', (1536,))
    P('sr1_w', (3, 256)); P('sr1_b', (256,))
    P('sr2_w', (256, 256)); P('sr2_b', (256,))
    P('sr3_w', (256, 1)); P('sr3_b', (1,))
    P('mq1_w', (1027, 1024)); P('mq1_b', (1024,))
    P('mq2_w', (1024, 1024)); P('mq2_b', (1024,))
    P('mq3_w', (1024, 384)); P('mq3_b', (384,))

    out_sel = nc.declare_dram_parameter('out_sel', [NB, NQ, 3], F32, isOutput=True)
    out_qf = nc.declare_dram_parameter('out_qf', [NB, NQ, 384], F32, isOutput=True)
    dbg = {}
    if DEBUG:
        for nm, sh in (('bank', [128, NRT * 3]), ('scores', [128, NRT]),
                       ('g1', [128, NRT * 256]), ('s2b', [128, NRT * 256]),
                       ('s3', [128, NRT]), ('gf', [128, 8 * NB]),
                       ('rank', [128, NRT]), ('selrows', [128, 48]),
                       ('x1', [128, 8 * 512]), ('sbc', [128, 4 * 768])):
            dbg[nm] = nc.declare_dram_parameter(f'dbg_{nm}', sh, F32, isOutput=True)

    # DRAM scratch for small repartition shuffles
    scr = nc.dram_tensor('scratch', [1 << 16], F32)           # 256KB
    scr2 = nc.dram_tensor('scratch2', [1 << 16], F32)

    with TileContext(nc) as tc:
        v = nc.vector
        gp = nc.gpsimd
        a = nc.scalar
        te = nc.tensor
        dma = nc.sync.dma_start

        import contextlib
        ctx = contextlib.ExitStack()
        # persistent pool (weights, long-lived intermediates)
        sbP = ctx.enter_context(tc.tile_pool(name='sbP', bufs=1))
        psA = ctx.enter_context(tc.tile_pool(name='psA', bufs=2, space='PSUM'))  # tag 'pb'
        psB = ctx.enter_context(tc.tile_pool(name='psB', bufs=1, space='PSUM'))  # aux tags

        # ---------- iotas / identity ----------
        def make_iota(shape, cm, tag):
            u = sbP.tile(shape, U32, tag=tag + '_u')
            gp.iota(u, [[1, shape[1]]], base=0, channel_multiplier=cm)
            f = sbP.tile(shape, F32, tag=tag)
            v.tensor_copy(f, u)
            return f
        ic_f = make_iota([128, 1], 1, 'ic')          # partition index
        it_f = make_iota([128, 128], 0, 'it')        # 0..127 along free
        ident = sbP.tile([128, 128], F32, tag='ident')
        v.tensor_scalar(ident, it_f, ic_f, None, AL.is_equal)
        iotaG = make_iota([128, 512], 512, 'iotaG')  # global point index p*512+f
        iota32r = make_iota([128, 32], 0, 'iota32r')
        iota768 = make_iota([128, 768], 0, 'iota768')
        iota512 = make_iota([128, 512], 0, 'iota512')

        # ---------- weight staging ----------
        def load_col8(name, n_tiles):
            tl = sbP.tile([128, n_tiles], F32, tag=f'c8_{name}')
            dma(out=tl, in_=ins[name][:].rearrange('(t p) -> p t', p=128))
            return tl
        conv1_bc = load_col8('conv1_b', 8)
        conv2_bc = load_col8('conv2_b', 8)
        bn_g8 = load_col8('bn_g', 8); bn_b8 = load_col8('bn_b', 8)
        bn_m8 = load_col8('bn_m', 8); bn_v8 = load_col8('bn_v', 8)
        cp1_bc = load_col8('cp1_b', 8)
        cp2_bc = load_col8('cp2_b', 12)
        mq1_bc = load_col8('mq1_b', 8)
        mq2_bc = load_col8('mq2_b', 8)

        bn_s8 = sbP.tile([128, 8], F32, tag='bn_s8')
        bn_t = sbP.tile([128, 8], F32, tag='bn_t')
        v.tensor_scalar(bn_t, bn_v8, 1e-5, None, AL.add)
        a.activation(bn_t, bn_t, AF.Sqrt)
        v.reciprocal(bn_s8, bn_t)
        v.tensor_mul(bn_s8, bn_g8, bn_s8)

        def stage_raw(name, rows, cols, pool=None):
            raw = (pool or sbP).tile([128, (rows // 128) * cols], F32, tag=f'raw_{name}')
            dma(out=raw.rearrange('p (t f) -> p t f', f=cols),
                in_=ins[name][:].rearrange('(t p) f -> p t f', p=128))
            return raw  # raw[p, t*cols+f] = w[128t+p, f]

        def stage_T(name, rows, cols, pool=None, raw=None):
            # produce wT[p, tc*rows + r] = w[r, 128*tc + p]
            nt_r, nt_c = rows // 128, cols // 128
            if raw is None:
                raw = stage_raw(name, rows, cols, pool)
            wT = (pool or sbP).tile([128, nt_c * rows], F32, tag=f'wT_{name}')
            for tr in range(nt_r):
                for tcol in range(nt_c):
                    pt = psA.tile([128, 128], F32, tag='pb')
                    te.transpose(pt, raw[:, tr * cols + tcol * 128: tr * cols + (tcol + 1) * 128], ident)
                    v.tensor_copy(wT[:, tcol * rows + tr * 128: tcol * rows + (tr + 1) * 128], pt)
            return wT

        sr2_w_r = stage_raw('sr2_w', 256, 256)
        sr1_w_rows = []
        for rr in range(3):
            t_r = sbP.tile([1, 256], F32, tag=f'sr1w{rr}')
            dma(out=t_r, in_=ins['sr1_w'][rr:rr + 1, :])
            sr1_w_rows.append(t_r)

        def load_row(name, n):
            tl = sbP.tile([1, n], F32, tag=f'r_{name}')
            dma(out=tl, in_=ins[name][:].rearrange('(o f) -> o f', o=1))
            return tl
        sr1_b_r = load_row('sr1_b', 256)
        sr2_b_r = load_row('sr2_b', 256)
        sr3_b_r = load_row('sr3_b', 1)
        mq3_b_r = load_row('mq3_b', 384)
        w3_row = sbP.tile([1, 256], F32, tag='w3row')
        dma(out=w3_row, in_=ins['sr3_w'][:].rearrange('(o k) c -> o (k c)', o=1))

        def bcast(src_row, n, tag):
            out = sbP.tile([128, n], F32, tag=tag)
            gp.partition_broadcast(out, src_row, channels=128)
            return out
        w1_b0 = bcast(sr1_w_rows[0], 256, 'w1b0')
        w1_b1 = bcast(sr1_w_rows[1], 256, 'w1b1')
        w1_b2 = bcast(sr1_w_rows[2], 256, 'w1b2')
        b1_bc = bcast(sr1_b_r, 256, 'b1bc')
        b2_bc = bcast(sr2_b_r, 256, 'b2bc')
        w3_bc = bcast(w3_row, 256, 'w3bc')
        mq3_b_bc = bcast(mq3_b_r, 384, 'mq3bbc')
        b3_col = sbP.tile([128, 1], F32, tag='b3col')
        gp.partition_broadcast(b3_col, sr3_b_r, channels=128)

        # batch one-hots and base offsets
        oh4 = sbP.tile([128, NB], F32, tag='oh4')       # oh4[p,b] = (p//32 == b)
        tmp4 = sbP.tile([128, NB], F32, tag='tmp4')
        bidx = make_iota([128, NB], 0, 'bidx')
        pq = sbP.tile([128, 1], F32, tag='pq')
        v.tensor_scalar(pq, ic_f, float(1.0 / 32.0), None, AL.mult)
        v.tensor_scalar(tmp4, bidx, pq, None, AL.subtract)     # b - p/32
        m14 = sbP.tile([128, NB], F32, tag='m14')
        v.tensor_scalar(m14, tmp4, 0.0, None, AL.is_le)
        v.tensor_scalar(tmp4, tmp4, -1.0, None, AL.is_gt)
        v.tensor_mul(oh4, m14, tmp4)
        oh4T = sbP.tile([128, 128], F32, tag='oh4T')
        oh4_pad = sbP.tile([128, 128], F32, tag='oh4pad')
        v.memset(oh4_pad, 0.0)
        v.tensor_copy(oh4_pad[:, 0:NB], oh4)
        ptt0 = psB.tile([128, 128], F32, tag='paux')
        te.transpose(ptt0, oh4_pad, ident)
        v.tensor_copy(oh4T, ptt0)
        base_col = sbP.tile([128, 1], F32, tag='base_col')
        v.memset(base_col, 0.0)
        for b in range(1, NB):
            v.scalar_tensor_tensor(base_col, oh4[:, b:b + 1], float(b * 16384.0), base_col,
                                   AL.mult, AL.add)

        # =====================================================
        # P0: conv chain -> gf_acc[p, 8b+t] = gf[b, 128t+p]
        # =====================================================
        gf_acc = sbP.tile([128, 8 * NB], F32, tag='gf_acc')
        v.memset(gf_acc, -1e30)
        PT = 512
        with tc.tile_pool(name='convp', bufs=2) as cp, \
             tc.tile_pool(name='convw', bufs=1) as cw:
            w1T = cw.tile([128, 3 * 1024], F32, tag='w1T')
            for tcq in range(3):
                dma(out=w1T[:, tcq * 1024:(tcq + 1) * 1024],
                    in_=ins['conv1_w'][:, tcq * 128:(tcq + 1) * 128].rearrange('r p -> p r'))
            w2T = cw.tile([128, 8 * 1024], F32, tag='w2T')
            for tcq in range(8):
                dma(out=w2T[:, tcq * 1024:(tcq + 1) * 1024],
                    in_=ins['conv2_w'][:, tcq * 128:(tcq + 1) * 128].rearrange('r p -> p r'))
            for b in range(NB):
                ef = cp.tile([128, 3 * 2048], F32, tag='ef')
                dma(out=ef.rearrange('p (t n) -> p t n', n=2048),
                    in_=ins['encoded_features'][b].rearrange('(t p) n -> p t n', p=128))
                for pc in range(2048 // PT):
                    xch = cp.tile([128, 8 * PT], F32, tag='xch')
                    for m in range(8):
                        pt = psA.tile([128, PT], F32, tag='pb')
                        for kc in range(3):
                            te.matmul(pt,
                                      w1T[:, kc * 1024 + m * 128: kc * 1024 + (m + 1) * 128],
                                      ef[:, kc * 2048 + pc * PT: kc * 2048 + pc * PT + PT],
                                      start=(kc == 0), stop=(kc == 2))
                        sl = slice(m * PT, (m + 1) * PT)
                        v.tensor_scalar(xch[:, sl], pt, conv1_bc[:, m:m + 1], None, AL.add)
                        v.tensor_scalar(xch[:, sl], xch[:, sl], bn_m8[:, m:m + 1], None, AL.subtract)
                        v.tensor_scalar(xch[:, sl], xch[:, sl], bn_s8[:, m:m + 1], bn_b8[:, m:m + 1],
                                        AL.mult, AL.add)
                        lrl = cp.tile([128, PT], F32, tag='lrl')
                        v.tensor_scalar(lrl, xch[:, sl], 0.2, None, AL.mult)
                        v.tensor_max(xch[:, sl], xch[:, sl], lrl)
                    if DEBUG and b == 0 and pc == 0:
                        dma(out=dbg['x1'][:], in_=xch)
                    for m2 in range(8):
                        pt2 = psA.tile([128, PT], F32, tag='pb')
                        for kc in range(8):
                            te.matmul(pt2,
                                      w2T[:, kc * 1024 + m2 * 128: kc * 1024 + (m2 + 1) * 128],
                                      xch[:, kc * PT:(kc + 1) * PT],
                                      start=(kc == 0), stop=(kc == 7))
                        cm = cp.tile([128, PT], F32, tag='cm')
                        v.tensor_scalar(cm, pt2, conv2_bc[:, m2:m2 + 1], None, AL.add)
                        cmx = cp.tile([128, 1], F32, tag='cmx')
                        v.tensor_reduce(cmx, cm, AX.X, AL.max)
                        v.tensor_max(gf_acc[:, 8 * b + m2: 8 * b + m2 + 1],
                                     gf_acc[:, 8 * b + m2: 8 * b + m2 + 1], cmx)
        if DEBUG:
            dma(out=dbg['gf'][:], in_=gf_acc)

        # =====================================================
        # P1: FPS
        # =====================================================
        fps_bank = sbP.tile([128, 256 * 3], F32, tag='fps_bank')  # rows 0..3 valid
        with tc.tile_pool(name='fpsp', bufs=1) as fp:
            planes = []
            for c in range(3):
                pl = fp.tile([128, 512], F32, tag=f'plane{c}')
                for b in range(NB):
                    dma(out=pl[32 * b:32 * (b + 1), :],
                        in_=ins['input_coords'][b, :, c].rearrange('(p f) -> p f', p=32))
                planes.append(pl)
            dists = fp.tile([128, 512], F32, tag='dists')
            v.memset(dists, 1e10)
            u1 = fp.tile([128, 512], F32, tag='u1')
            u2 = fp.tile([128, 512], F32, tag='u2')
            u3 = fp.tile([128, 512], F32, tag='u3')
            m2t = fp.tile([128, 512], F32, tag='m2t')
            negc = fp.tile([128, 3], F32, tag='negc')
            sxyz = fp.tile([128, 3], F32, tag='sxyz')
            cst = fp.tile([128, 3], F32, tag='cst')
            mv8 = fp.tile([128, 8], F32, tag='mv8')
            mi8 = fp.tile([128, 8], U32, tag='mi8')
            mi8f = fp.tile([128, 1], F32, tag='mi8f')
            mv2 = fp.tile([128, 8], F32, tag='mv2')
            mi2 = fp.tile([128, 8], U32, tag='mi2')
            mi2f = fp.tile([128, 1], F32, tag='mi2f')
            msk32 = fp.tile([128, 32], F32, tag='msk32')
            fi_col = fp.tile([128, 1], F32, tag='fi_col')
            gval = fp.tile([128, 1], F32, tag='gval')
            g_col = fp.tile([128, 1], F32, tag='g_col')
            stage_in = fp.tile([128, 32], F32, tag='stage_in')
            v.memset(stage_in, -1e30)
            stage_out = fp.tile([128, 32], F32, tag='stage_out')
            stage2_in = fp.tile([128, 32], F32, tag='stage2_in')
            v.memset(stage2_in, 0.0)
            stage2_out = fp.tile([128, 32], F32, tag='stage2_out')
            grow = fp.tile([128, 32], F32, tag='grow')
            gcolT = fp.tile([128, 32], F32, tag='gcolT')
            v.tensor_copy(g_col, base_col)     # initial winner: point 0 per batch
            pred = psB.tile([NB, 3], F32, tag='pred')
            pbc = psB.tile([128, 3], F32, tag='pbc')

            for it in range(FPS_ITERS):
                # gather centroid from g_col
                v.tensor_scalar(m2t, iotaG, g_col, None, AL.is_equal)
                v.scalar_tensor_tensor(u1, planes[0], 1.0, m2t, AL.mult, AL.mult,
                                       accum_out=sxyz[:, 0:1])
                v.scalar_tensor_tensor(u2, planes[1], 1.0, m2t, AL.mult, AL.mult,
                                       accum_out=sxyz[:, 1:2])
                v.scalar_tensor_tensor(u3, planes[2], 1.0, m2t, AL.mult, AL.mult,
                                       accum_out=sxyz[:, 2:3])
                te.matmul(pred, oh4, sxyz, start=True, stop=True)
                v.tensor_copy(cst[0:NB, :], pred)
                v.tensor_copy(fps_bank[0:NB, it * 3:(it + 1) * 3], cst[0:NB, :])
                v.tensor_scalar(cst[0:NB, :], cst[0:NB, :], -1.0, None, AL.mult)
                te.matmul(pbc, oh4T[0:NB, :], cst[0:NB, :], start=True, stop=True)
                v.tensor_copy(negc, pbc)
                # distance update (bit-exact)
                a.activation(u1, planes[0], AF.Square, bias=negc[:, 0:1], scale=1.0)
                a.activation(u2, planes[1], AF.Square, bias=negc[:, 1:2], scale=1.0)
                a.activation(u3, planes[2], AF.Square, bias=negc[:, 2:3], scale=1.0)
                v.tensor_add(u1, u1, u2)
                v.tensor_add(u1, u1, u3)
                v.tensor_tensor(dists, dists, u1, AL.min)
                # argmax (first occurrence, per batch)
                v.max(mv8, dists)
                v.max_index(mi8, mv8, dists)
                v.tensor_copy(mi8f, mi8[:, 0:1])
                v.tensor_copy(stage_in[:, 0:1], mv8[:, 0:1])
                v.transpose(stage_out, stage_in)
                v.max(mv2, stage_out)
                v.max_index(mi2, mv2, stage_out)
                v.tensor_copy(mi2f, mi2[:, 0:1])
                v.tensor_copy(stage2_in[:, 0:1], mi8f)
                v.transpose(stage2_out, stage2_in)
                v.tensor_scalar(msk32, iota32r, mi2f, None, AL.is_equal)
                v.tensor_mul(msk32, msk32, stage2_out)
                v.tensor_reduce(fi_col, msk32, AX.X, AL.add)
                v.tensor_scalar(gval, mi2f, 512.0, None, AL.mult)
                v.tensor_add(gval, gval, fi_col)
                v.tensor_add(gval, gval, base_col)
                v.tensor_copy(grow, gval.to_broadcast([128, 32]))
                v.transpose(gcolT, grow)
                v.tensor_copy(g_col, gcolT[:, 0:1])

        # =====================================================
        # P2: cp chain
        # =====================================================
        gf_r = sbP.tile([128, 8 * NB], F32, tag='gf_r')   # gf_r[p, kc*NB+b]
        for t in range(8):
            v.tensor_copy(gf_r[:, t * NB:(t + 1) * NB], gf_acc[:, t::8])
        coarseT = sbP.tile([128, 12 * NB], F32, tag='coarseT')
        with tc.tile_pool(name='cpp', bufs=1) as cpl:
            cp1_w_r = stage_raw('cp1_w', 1024, 1024, pool=cpl)
            cp2_w_r = stage_raw('cp2_w', 1024, 1536, pool=cpl)
            hT = cpl.tile([128, 8 * NB], F32, tag='hT')
            for m in range(8):
                pt = psA.tile([128, NB], F32, tag='pb')
                for kc in range(8):
                    te.matmul(pt,
                              cp1_w_r[:, kc * 1024 + m * 128: kc * 1024 + (m + 1) * 128],
                              gf_r[:, kc * NB:(kc + 1) * NB],
                              start=(kc == 0), stop=(kc == 7))
                v.tensor_scalar(hT[:, m * NB:(m + 1) * NB], pt, cp1_bc[:, m:m + 1], None, AL.add)
                v.tensor_relu(hT[:, m * NB:(m + 1) * NB], hT[:, m * NB:(m + 1) * NB])
            for m in range(12):
                pt = psA.tile([128, NB], F32, tag='pb')
                for kc in range(8):
                    te.matmul(pt,
                              cp2_w_r[:, kc * 1536 + m * 128: kc * 1536 + (m + 1) * 128],
                              hT[:, kc * NB:(kc + 1) * NB],
                              start=(kc == 0), stop=(kc == 7))
                v.tensor_scalar(coarseT[:, m * NB:(m + 1) * NB], pt, cp2_bc[:, m:m + 1],
                                None, AL.add)

        # =====================================================
        # P3: bank assembly via DRAM bounce
        # coarseT[p, m*NB+b] = coarse_flat[b, 128m+p]; coarse_flat[b, 3q+c]
        # bank_rows[p, t*3+c], bank row r = 128t+p = b*768 + q(+512 for fps)
        # =====================================================
        bank_rows = sbP.tile([128, NRT * 3], F32, tag='bank_rows')
        # write coarseT to scratch in [b, 1536] layout: scr[b*1536 + 128m+p]
        _w1 = []
        for b in range(NB):
            _w1.append(dma(out=scr[b * 1536:(b + 1) * 1536].rearrange('(m p) -> p m', p=128),
                in_=coarseT[:, b::NB]))
        # fps_bank rows 0..3 [4, 768] -> scr2[b*768 + 3s+c]
        _w2 = dma(out=scr2[0:NB * 768].rearrange('(b f) -> b f', b=NB), in_=fps_bank[0:NB, :])
        # read back into bank_rows tiles
        for b in range(NB):
            for toff in range(4):
                t = b * 6 + toff
                _r = dma(out=bank_rows[:, t * 3:(t + 1) * 3],
                    in_=scr[b * 1536 + toff * 384: b * 1536 + (toff + 1) * 384]
                        .rearrange('(p c) -> p c', c=3))
                for _w in _w1:
                    add_dep_helper(_r.ins, _w.ins, sync=True, reason='scr bounce')
            for toff in range(2):
                t = b * 6 + 4 + toff
                _r = dma(out=bank_rows[:, t * 3:(t + 1) * 3],
                    in_=scr2[b * 768 + toff * 384: b * 768 + (toff + 1) * 384]
                        .rearrange('(p c) -> p c', c=3))
                add_dep_helper(_r.ins, _w2.ins, sync=True, reason='scr2 bounce')
        if DEBUG:
            dma(out=dbg['bank'][:], in_=bank_rows)

        # =====================================================
        # P4: score MLP (bit-exact mimicry)
        # =====================================================
        scores = sbP.tile([128, NRT], F32, tag='scores')
        with tc.tile_pool(name='scp', bufs=1) as sp:
            g1 = sp.tile([128, NRT * 256], F32, tag='g1')
            scrA = sp.tile([128, NRT * 64], F32, tag='scrA')
            scrB = sp.tile([128, NRT * 64], F32, tag='scrB')
            scrC = sp.tile([128, NRT * 64], F32, tag='scrC')

            def gelu_emit(x_full, out_full, z_f, w_f, p_f, n=NRT * 256):
                nch = n // (NRT * 64)
                for ci in range(nch):
                    s = slice(ci * NRT * 64, (ci + 1) * NRT * 64)
                    x = x_full[:, s]; out = out_full[:, s]
                    z = z_f; w = w_f; p = p_f
                    v.tensor_scalar(z, x, float(-INV_SQRT2), None, AL.mult)
                    v.tensor_mul(w, z, z)
                    v.tensor_scalar(p, w, float(ERFC_C[0]), float(ERFC_C[1]), AL.mult, AL.add)
                    for c in ERFC_C[2:]:
                        v.tensor_mul(p, p, w)
                        v.tensor_scalar(p, p, float(c), None, AL.add)
                    v.tensor_mul(p, z, p)
                    v.tensor_scalar(p, p, -1.0, 1.0, AL.mult, AL.add)
                    v.tensor_scalar(z, x, 0.5, None, AL.mult)
                    v.tensor_mul(out, z, p)

            def veltkamp(src_f, hi_f, lo_f, tmp, chunk=NRT * 64):
                n = src_f.shape[1]
                for ci in range(max(1, n // chunk)):
                    s = slice(ci * chunk, min(n, (ci + 1) * chunk))
                    src = src_f[:, s]; hi = hi_f[:, s]; lo = lo_f[:, s]
                    tm = tmp[:, 0:(s.stop - s.start)]
                    v.tensor_scalar(tm, src, 4097.0, None, AL.mult)
                    v.tensor_sub(hi, tm, src)
                    v.tensor_sub(hi, tm, hi)
                    v.tensor_sub(lo, src, hi)

            # s1 + b1
            for t in range(NRT):
                sl = slice(t * 256, (t + 1) * 256)
                sa = scrA[:, (t % 6) * 256:((t % 6) + 1) * 256]
                v.tensor_scalar(g1[:, sl], w1_b0, bank_rows[:, t * 3:t * 3 + 1], None, AL.mult)
                v.scalar_tensor_tensor(sa, w1_b1, bank_rows[:, t * 3 + 1:t * 3 + 2],
                                       g1[:, sl], AL.mult, AL.add)
                v.scalar_tensor_tensor(g1[:, sl], w1_b2, bank_rows[:, t * 3 + 2:t * 3 + 3],
                                       sa, AL.mult, AL.add)
                v.tensor_add(g1[:, sl], g1[:, sl], b1_bc)
            gelu_emit(g1, g1, scrA, scrB, scrC)
            if DEBUG:
                dma(out=dbg['g1'][:], in_=g1)

            # s2 on PE, 12-bit splits (simulable)
            g1T = sp.tile([128, 2 * NROWS], F32, tag='g1T')
            for t in range(NRT):
                for kc in range(2):
                    pt = psA.tile([128, 128], F32, tag='pb')
                    te.transpose(pt, g1[:, t * 256 + kc * 128: t * 256 + (kc + 1) * 128], ident)
                    v.tensor_copy(g1T[:, kc * NROWS + t * 128: kc * NROWS + (t + 1) * 128], pt)
            g1Th = sp.tile([128, 2 * NROWS], F32, tag='g1Th')
            g1Tl = sp.tile([128, 2 * NROWS], F32, tag='g1Tl')
            veltkamp(g1T, g1Th, g1Tl, scrA)  # chunked internally
            w2h = sbP.tile([128, 2 * 256], F32, tag='w2h')
            w2l = sbP.tile([128, 2 * 256], F32, tag='w2l')
            w2t = sbP.tile([128, 2 * 256], F32, tag='w2t')
            veltkamp(sr2_w_r, w2h, w2l, w2t)

            s2b = g1T  # reuse slot (g1T dead after splits)... NOT dead: splits read g1T
            s2b = sp.tile([128, NRT * 256], F32, tag='s2b')
            for t in range(NRT):
                pt = psA.tile([128, 256], F32, tag='pb')
                seq = [(g1Th, w2h), (g1Th, w2l), (g1Tl, w2h), (g1Tl, w2l)]
                n_mm = 0
                for kc in range(2):
                    for (lh, rh) in seq:
                        te.matmul(pt,
                                  lh[:, kc * NROWS + t * 128: kc * NROWS + (t + 1) * 128],
                                  rh[:, kc * 256:(kc + 1) * 256],
                                  start=(n_mm == 0), stop=(n_mm == 7))
                        n_mm += 1
                v.tensor_tensor(s2b[:, t * 256:(t + 1) * 256], pt, b2_bc, AL.add)
            if DEBUG:
                dma(out=dbg['s2b'][:], in_=s2b)

            g2 = g1  # reuse g1 slot (g1 dead after transpose)
            gelu_emit(s2b, g2, scrA, scrB, scrC)

            # s3: exact FMA-lane-chain emulation
            g2h = g1T   # reuse
            g2l = g1Th  # reuse (g1Th dead after s2 matmuls)
            veltkamp(g2, g2h[:, 0:NRT * 256], g2l[:, 0:NRT * 256], scrA)
            w3h = sbP.tile([128, 256], F32, tag='w3h')
            w3l = sbP.tile([128, 256], F32, tag='w3l')
            w3t = sbP.tile([128, 256], F32, tag='w3t')
            veltkamp(w3_bc, w3h, w3l, w3t)

            accL = sp.tile([128, NRT * 16], F32, tag='accL')
            v.memset(accL, 0.0)
            t_hh = sp.tile([128, NRT * 16], F32, tag='t_hh')
            t_c = sp.tile([128, NRT * 16], F32, tag='t_c')
            t_s = sp.tile([128, NRT * 16], F32, tag='t_s')
            t_bb = sp.tile([128, NRT * 16], F32, tag='t_bb')
            t_e1 = sp.tile([128, NRT * 16], F32, tag='t_e1')
            t_e2 = sp.tile([128, NRT * 16], F32, tag='t_e2')
            for j in range(16):
                whj = w3h[:, j * 16:(j + 1) * 16]
                wlj = w3l[:, j * 16:(j + 1) * 16]
                for t in range(NRT):
                    gh = g2h[:, t * 256 + j * 16: t * 256 + (j + 1) * 16]
                    gl = g2l[:, t * 256 + j * 16: t * 256 + (j + 1) * 16]
                    sl = slice(t * 16, (t + 1) * 16)
                    v.tensor_mul(t_hh[:, sl], gh, whj)
                    v.tensor_mul(t_c[:, sl], gh, wlj)
                    v.tensor_mul(t_e1[:, sl], gl, whj)
                    v.tensor_add(t_c[:, sl], t_c[:, sl], t_e1[:, sl])
                    v.tensor_mul(t_e1[:, sl], gl, wlj)
                    v.tensor_add(t_c[:, sl], t_c[:, sl], t_e1[:, sl])
                    v.tensor_add(t_s[:, sl], accL[:, sl], t_hh[:, sl])
                    v.tensor_sub(t_bb[:, sl], t_s[:, sl], accL[:, sl])
                    v.tensor_sub(t_e2[:, sl], t_s[:, sl], t_bb[:, sl])
                    v.tensor_sub(t_e2[:, sl], accL[:, sl], t_e2[:, sl])
                    v.tensor_sub(t_e1[:, sl], t_hh[:, sl], t_bb[:, sl])
                    v.tensor_add(t_e2[:, sl], t_e2[:, sl], t_e1[:, sl])
                    v.tensor_add(t_e2[:, sl], t_e2[:, sl], t_c[:, sl])
                    v.tensor_add(accL[:, sl], t_s[:, sl], t_e2[:, sl])
            s3 = sbP.tile([128, NRT], F32, tag='s3')
            accR = accL.rearrange('p (t k) -> p t k', k=16)
            binr = sp.tile([128, NRT * 8], F32, tag='binr')
            binrR = binr.rearrange('p (t k) -> p t k', k=8)
            v.tensor_add(binrR, accR[:, :, 8:16], accR[:, :, 0:8])
            tt1 = sp.tile([128, NRT * 4], F32, tag='tt1')
            tt1R = tt1.rearrange('p (t k) -> p t k', k=4)
            v.tensor_add(tt1R, binrR[:, :, 0:4], binrR[:, :, 4:8])
            tt2 = sp.tile([128, NRT * 2], F32, tag='tt2')
            tt2R = tt2.rearrange('p (t k) -> p t k', k=2)
            v.tensor_add(tt2R, tt1R[:, :, 0:2], tt1R[:, :, 2:4])
            v.tensor_add(s3.rearrange('p (t k) -> p t k', k=1),
                         tt2R[:, :, 0:1], tt2R[:, :, 1:2])
            if DEBUG:
                dma(out=dbg['s3'][:], in_=s3)

            # sigmoid
            xt = sp.tile([128, NRT], F32, tag='xt')
            pt_ = sp.tile([128, NRT], F32, tag='pt_')
            zt = sp.tile([128, NRT], F32, tag='zt')
            v.tensor_scalar(xt, s3, b3_col, None, AL.add)
            v.tensor_scalar(xt, xt, -1.0, None, AL.mult)
            v.tensor_scalar(pt_, xt, float(EXP_CS[0]), None, AL.mult)
            v.tensor_scalar(pt_, pt_, float(EXP_CS[1]), None, AL.add)
            for c in EXP_CS[2:]:
                v.tensor_mul(pt_, pt_, xt)
                v.tensor_scalar(pt_, pt_, float(c), None, AL.add)
            v.tensor_mul(pt_, pt_, xt)
            v.tensor_scalar(pt_, pt_, 0.5, None, AL.add)
            v.tensor_mul(zt, xt, xt)
            v.tensor_mul(pt_, pt_, zt)
            v.tensor_add(pt_, xt, pt_)
            v.tensor_scalar(pt_, pt_, 1.0, None, AL.add)
            v.tensor_scalar(pt_, pt_, 1.0, None, AL.add)
            v.reciprocal(scores, pt_)
        if DEBUG:
            dma(out=dbg['scores'][:], in_=scores)

        # =====================================================
        # P5: ranking + gathers
        # =====================================================
        rank = sbP.tile([128, NRT], F32, tag='rank')
        selT = sbP.tile([3, NB * 512], F32, tag='selT')
        sel_rows = sbP.tile([128, 16 * 3], F32, tag='sel_rows')
        with tc.tile_pool(name='rkp', bufs=1) as rp:
            # score rows [NB, 768] via DRAM bounce: scr[b*768 + q], q=128*tl+p
            _ws = dma(out=scr[0:NROWS].rearrange('(t p) -> p t', p=128), in_=scores)
            sbc = rp.tile([128, NB * 768], F32, tag='sbc')
            for b in range(NB):
                s_rowB = rp.tile([1, 768], F32, tag=f's_rowB{b}')
                _r = dma(out=s_rowB, in_=scr[b * 768:(b + 1) * 768].rearrange('(o f) -> o f', o=1))
                add_dep_helper(_r.ins, _ws.ins, sync=True, reason='scores bounce')
                gp.partition_broadcast(sbc[:, b * 768:(b + 1) * 768], s_rowB,
                                       channels=128)
            qidx = rp.tile([128, NRT], F32, tag='qidx')
            for t in range(NRT):
                v.tensor_scalar(qidx[:, t:t + 1], ic_f, float(128 * (t % 6)), None, AL.add)
            cmp_t = rp.tile([128, 768], F32, tag='cmp_t')
            cmp_e = rp.tile([128, 768], F32, tag='cmp_e')
            tie_c = rp.tile([128, 1], F32, tag='tie_c')
            junk = rp.tile([128, 768], F32, tag='junk')
            for t in range(NRT):
                b = t // 6
                ssl = sbc[:, b * 768:(b + 1) * 768]
                v.tensor_scalar(cmp_t, ssl, scores[:, t:t + 1], None, AL.is_gt)
                v.tensor_reduce(rank[:, t:t + 1], cmp_t, AX.X, AL.add)
                v.tensor_scalar(cmp_e, ssl, scores[:, t:t + 1], None, AL.is_equal)
                v.tensor_scalar(cmp_t, iota768, qidx[:, t:t + 1], None, AL.is_lt)
                v.tensor_mul(cmp_e, cmp_e, cmp_t)
                v.tensor_reduce(tie_c, cmp_e, AX.X, AL.add)
                v.tensor_add(rank[:, t:t + 1], rank[:, t:t + 1], tie_c)
            if DEBUG:
                dma(out=dbg['sbc'][:], in_=sbc)
            if DEBUG:
                dma(out=dbg['rank'][:], in_=rank)

            oh_t = rp.tile([128, 512], F32, tag='oh_t')
            for b in range(NB):
                ps_selT = psB.tile([3, 512], F32, tag='pselT')
                for tloc in range(6):
                    t = b * 6 + tloc
                    v.tensor_scalar(oh_t, iota512, rank[:, t:t + 1], None, AL.is_equal)
                    te.matmul(ps_selT, bank_rows[:, t * 3:(t + 1) * 3], oh_t,
                              start=(tloc == 0), stop=(tloc == 5))
                v.tensor_copy(selT[:, b * 512:(b + 1) * 512], ps_selT)
            for b in range(NB):
                for mc in range(4):
                    ps_selR = psA.tile([128, 3], F32, tag='pb')
                    for tloc in range(6):
                        t = b * 6 + tloc
                        v.tensor_scalar(oh_t[:, 0:128], iota512[:, mc * 128:(mc + 1) * 128],
                                        rank[:, t:t + 1], None, AL.is_equal)
                        te.matmul(ps_selR, oh_t[:, 0:128], bank_rows[:, t * 3:(t + 1) * 3],
                                  start=(tloc == 0), stop=(tloc == 5))
                    ot = b * 4 + mc
                    v.tensor_copy(sel_rows[:, ot * 3:(ot + 1) * 3], ps_selR)
        dma(out=out_sel[:].rearrange('b (ot p) c -> p b ot c', p=128),
            in_=sel_rows.rearrange('p (b ot c) -> p b ot c', b=NB, ot=4))
        if DEBUG:
            dma(out=dbg['selrows'][:], in_=sel_rows)

        # =====================================================
        # P6: mq MLP (value path)
        # =====================================================
        with tc.tile_pool(name='mqp', bufs=1) as mp:
            mq1_w_r = mp.tile([128, 8 * 1024], F32, tag='raw_mq1_w')
            dma(out=mq1_w_r.rearrange('p (t f) -> p t f', f=1024),
                in_=ins['mq1_w'][0:1024, :].rearrange('(t p) f -> p t f', p=128))
            mq1_w_tail = mp.tile([3, 1024], F32, tag='mq1_tail')
            dma(out=mq1_w_tail, in_=ins['mq1_w'][1024:1027, :])
            mq2_w_r = stage_raw('mq2_w', 1024, 1024, pool=mp)
            mq3_w_r = stage_raw('mq3_w', 1024, 384, pool=mp)
            q1o = mp.tile([128, 8 * 512], F32, tag='q1o')
            q2o = mp.tile([128, 8 * 512], F32, tag='q2o')
            rhs_gfb = mp.tile([128, 8 * 512], F32, tag='rhs_gfb')
            for b in range(NB):
                for kc in range(8):
                    v.tensor_copy(rhs_gfb[:, kc * 512:(kc + 1) * 512],
                                  gf_r[:, kc * NB + b: kc * NB + b + 1].to_broadcast([128, 512]))
                for m in range(8):
                    pt = psA.tile([128, 512], F32, tag='pb')
                    for kc in range(8):
                        te.matmul(pt,
                                  mq1_w_r[:, kc * 1024 + m * 128: kc * 1024 + (m + 1) * 128],
                                  rhs_gfb[:, kc * 512:(kc + 1) * 512],
                                  start=(kc == 0), stop=False)
                    te.matmul(pt, mq1_w_tail[:, m * 128:(m + 1) * 128],
                              selT[:, b * 512:(b + 1) * 512], start=False, stop=True)
                    a.activation(q1o[:, m * 512:(m + 1) * 512], pt,
                                 AF.Gelu, bias=mq1_bc[:, m:m + 1], scale=1.0)
                for m in range(8):
                    pt = psA.tile([128, 512], F32, tag='pb')
                    for kc in range(8):
                        te.matmul(pt,
                                  mq2_w_r[:, kc * 1024 + m * 128: kc * 1024 + (m + 1) * 128],
                                  q1o[:, kc * 512:(kc + 1) * 512],
                                  start=(kc == 0), stop=(kc == 7))
                    a.activation(q2o[:, m * 512:(m + 1) * 512], pt,
                                 AF.Gelu, bias=mq2_bc[:, m:m + 1], scale=1.0)
                for rtl in range(4):
                    pt = psA.tile([128, 384], F32, tag='pb')
                    for kc in range(8):
                        te.matmul(pt,
                                  q2o[:, kc * 512 + rtl * 128: kc * 512 + (rtl + 1) * 128],
                                  mq3_w_r[:, kc * 384:(kc + 1) * 384],
                                  start=(kc == 0), stop=(kc == 7))
                    qf_t = mp.tile([128, 384], F32, tag='qf_t')
                    v.tensor_tensor(qf_t, pt, mq3_b_bc, AL.add)
                    dma(out=out_qf[:].rearrange('b (rt p) f -> b rt p f', p=128)[b, rtl], in_=qf_t)

        ctx.close()
    return nc


_CACHE = {}


def _get_nc():
    if 'nc' not in _CACHE:
        nc = bacc.Bacc('TRN2', target_bir_lowering=False, debug=True)
        build(nc)
        nc.compile()
        _CACHE['nc'] = nc
    return _CACHE['nc']


def kernel(**inputs):
    nc = _get_nc()
    B = inputs['encoded_features'].shape[0]
    ncores = 8
    bpc = B // ncores
    wn = [k for k in inputs if k not in ('encoded_features', 'input_coords')]
    in_maps = []
    for c in range(ncores):
        m = {'encoded_features': np.ascontiguousarray(inputs['encoded_features'][c * bpc:(c + 1) * bpc], dtype=np.float32),
             'input_coords': np.ascontiguousarray(inputs['input_coords'][c * bpc:(c + 1) * bpc], dtype=np.float32)}
        for w in wn:
            m[w] = np.ascontiguousarray(inputs[w], dtype=np.float32)
        in_maps.append(m)
    res = run_bass_kernel_spmd(nc, in_maps, core_ids=list(range(ncores)))
    sel = np.concatenate([res.results[c]['out_sel'] for c in range(ncores)], axis=0)
    qf = np.concatenate([res.results[c]['out_qf'] for c in range(ncores)], axis=0)
    if DEBUG:
        kernel.debug = res.results
    return sel, qf
